# revision 23
# baseline (speedup 1.0000x reference)
"""Single-head attention (batch=4, seq=2048, d=1024) on 8 Trainium2 cores.

Sharding: 2D — data-parallel over batch (4) x query-block parallel (2).
Core c handles batch b = c // 2, query half q = c % 2. Each core receives
its batch's X^T with columns rolled so its query block is always columns
0..1023 (keys are order-invariant under softmax; the host un-rolls the
returned attention-weight columns).

Host-side packing (layout + dtype only, no FLOPs): inputs are shipped as
bf16, X and Wv pre-transposed, halving the input DMA bytes and removing
all on-device input transposes. All compute (5 chained matmuls, softmax)
runs on the NeuronCores.

Math per core (queries m = 0..1023, keys n = 0..2047):
    G  = Wq^T @ Wk                (natural layouts)
    Tt = G^T @ X^T                ([d_model, m] layout)
    A  = Tt^T @ Xt                (scores, [m, n]; Q@K^T == X G X^T)
    E  = exp(A / sqrt(dk))        (no max-subtraction: |A/32| < ~2.5)
    w  = E * (1/rowsum(E))        (fp32, streamed out)
    V  = Xt^T @ Wv^T              ([n, v] layout)
    ctx = (E^T)^T @ V * (1/rowsum)

Matmuls run in bf16 with fp32 PSUM accumulation; the weights output keeps
full fp32 exp/normalize precision. E^T rides the DMA engines' XBAR
transpose. Matmul loops are stationary-outer so a post-pass can elide
redundant LDWEIGHTS (the PE retains its stationary operand).
"""

import sys

sys.path.insert(0, "/opt/trn_rl_repo")

import ml_dtypes
import numpy as np

import concourse.bass as bass
import concourse.mybir as mybir
import concourse.tile as tile
from concourse.bass_utils import run_bass_kernel_spmd

P = 128
S = 2048  # sequence length (keys)
M = 1024  # queries per core
D = 1024  # d_model == d_k == d_v
SC = S // P  # 16 key chunks
MC = M // P  # 8 query chunks
DC = D // P  # 8 feature chunks
SCALE = 1.0 / np.sqrt(np.float32(D))

F32 = mybir.dt.float32
BF16 = mybir.dt.bfloat16
NP_BF16 = ml_dtypes.bfloat16

# This walrus build encodes at most one semaphore wait per instruction
# (its TPB_CTRL lowering rejects more with "Too many sync wait commands").
MAX_WAITS = 1


def _split_excess_waits(nc):
    for f in nc.m.functions:
        for blk in f.blocks:
            new_insts = []
            for inst in blk.instructions:
                si = inst.sync_info
                if si is not None and si.on_wait and len(si.on_wait) > MAX_WAITS:
                    waits = list(si.on_wait)
                    excess, keep = waits[:-MAX_WAITS], waits[-MAX_WAITS:]
                    for i in range(0, len(excess), MAX_WAITS):
                        chunk = excess[i : i + MAX_WAITS]
                        nop = mybir.InstNoOp(
                            name=f"{inst.name}-waitsplit-{i}",
                            ins=[],
                            outs=[],
                            sync_info=mybir.SyncInfo(on_wait=chunk, on_update=[]),
                        )
                        nop.engine = inst.engine
                        new_insts.append(nop)
                    si.on_wait = keep
                new_insts.append(inst)
            blk.instructions[:] = new_insts


def _dedupe_ldweights(nc):
    """Drop/neuter an InstLdweights whose weights AP equals the previously
    loaded one with no intervening PE weight clobber. The PE array retains
    the stationary operand across matmuls, so the reload is pure overhead
    (verified on hardware)."""
    n = 0
    for f in nc.m.functions:
        for blk in f.blocks:
            last_key = None
            for i, inst in enumerate(blk.instructions):
                nm = type(inst).__name__
                if nm == "InstLdweights":
                    try:
                        key = str(inst.ins[0])
                    except Exception:
                        key = None
                    if key is not None and key == last_key:
                        si = inst.sync_info
                        if si is None or (not si.on_wait and not si.on_update):
                            blk.instructions[i] = None  # drop entirely
                        else:
                            nop = mybir.InstNoOp(
                                name=inst.name + "-ldwdedup",
                                ins=[],
                                outs=[],
                                sync_info=si,
                            )
                            nop.engine = inst.engine
                            blk.instructions[i] = nop
                        n += 1
                    else:
                        last_key = key
                elif nm == "InstMatmult":
                    if getattr(inst, "is_transpose", False):
                        last_key = None
                elif nm in ("InstNoOp", "InstEventSemaphore"):
                    pass
                elif inst.engine == mybir.EngineType.PE:
                    last_key = None
            blk.instructions[:] = [i for i in blk.instructions if i is not None]
    return n


def _build_program():
    nc = bass.Bass()

    # All inputs pre-packed on the host: bf16, X and Wv transposed.
    xt = nc.dram_tensor("xt", [D, S], BF16, kind="ExternalInput")  # X^T
    wqb = nc.dram_tensor("wqb", [D, D], BF16, kind="ExternalInput")  # [d, c]
    wkb = nc.dram_tensor("wkb", [D, D], BF16, kind="ExternalInput")  # [d, c]
    wvt = nc.dram_tensor("wvt", [D, D], BF16, kind="ExternalInput")  # Wv^T [c, v]
    w_out = nc.dram_tensor("w_out", [M, S], F32, kind="ExternalOutput")
    ctx_out = nc.dram_tensor("ctx_out", [M, D], F32, kind="ExternalOutput")

    with tile.TileContext(nc) as tc:
        with (
            tc.tile_pool(name="pxt", bufs=1) as pxt,
            tc.tile_pool(name="pw8", bufs=3) as pw8,
            tc.tile_pool(name="pwv", bufs=1) as pwv,
            tc.tile_pool(name="pv", bufs=1) as pv,
            tc.tile_pool(name="pet", bufs=1) as pet,
            tc.tile_pool(name="pe32", bufs=2) as pe32,
            tc.tile_pool(name="pebf", bufs=2) as pebf,
            tc.tile_pool(name="pcst", bufs=2) as pcst,
            tc.tile_pool(name="psmall", bufs=1) as psmall,
            tc.tile_pool(name="ptiny", bufs=4) as ptiny,
            tc.tile_pool(name="ppsa", bufs=4, space="PSUM") as ppsa,
        ):
            r_all = psmall.tile([P, MC], F32, tag="rall")

            # Persistent bf16 tensors.
            Xt = pxt.tile([P, DC, S], BF16, tag="xt")  # X^T: [c, n]
            Wqb = pw8.tile([P, DC, D], BF16, tag="w8")  # natural [d, c]
            Wkb = pw8.tile([P, DC, D], BF16, tag="w8")
            WvT = pwv.tile([P, DC, D], BF16, tag="wv")  # [c, v]
            V = pv.tile([P, SC, D], BF16, tag="v")  # [n, v]
            Et = pet.tile([P, SC, M], BF16, tag="et")  # exp(A)^T: [n, m]

            # ---- Loads, ordered by when the PE needs them.
            for di in range(DC):
                nc.sync.dma_start(
                    out=Wqb[:, di, :], in_=wqb[di * P : (di + 1) * P, :]
                )
                nc.sync.dma_start(
                    out=Wkb[:, di, :], in_=wkb[di * P : (di + 1) * P, :]
                )

            # ---- G = Wq^T @ Wk  -> bf16 [c, c'] natural layout.
            # di-outer over groups of 4 ci-chunks (4 PSUM tiles = all 8
            # banks) so the matmuls start as soon as the first wq/wk
            # d-chunks land instead of waiting for the full 4 MB.
            G = pw8.tile([P, DC, D], BF16, tag="w8")
            for cig in range(2):
                pss = [
                    ppsa.tile([P, D], F32, tag="psa", name=f"psG{cig}_{cil}")
                    for cil in range(4)
                ]
                for di in range(DC):
                    for cil in range(4):
                        ci = cig * 4 + cil
                        for nt in range(2):
                            nc.tensor.matmul(
                                pss[cil][:, nt * 512 : (nt + 1) * 512],
                                Wqb[:, di, ci * P : (ci + 1) * P],
                                Wkb[:, di, nt * 512 : (nt + 1) * 512],
                                start=(di == 0),
                                stop=(di == DC - 1),
                                skip_group_check=True,
                            )
                for cil in range(4):
                    nc.vector.tensor_copy(
                        out=G[:, cig * 4 + cil, :], in_=pss[cil]
                    )

            nc.gpsimd.dma_start(out=Xt, in_=xt.rearrange("(a p) s -> p a s", p=P))
            nc.sync.dma_start(out=WvT, in_=wvt.rearrange("(a p) v -> p a v", p=P))

            # ---- Tt = G^T @ X^T  -> bf16 [c', m].
            Tt = pw8.tile([P, DC, M], BF16, tag="w8")
            for cj in range(DC):
                ps = ppsa.tile([P, D], F32, tag="psa", name=f"psT{cj}")
                for ci in range(DC):
                    for mt in range(2):
                        nc.tensor.matmul(
                            ps[:, mt * 512 : (mt + 1) * 512],
                            G[:, ci, cj * P : (cj + 1) * P],
                            Xt[:, ci, mt * 512 : (mt + 1) * 512],
                            start=(ci == 0),
                            stop=(ci == DC - 1),
                            skip_group_check=True,
                        )
                nc.vector.tensor_copy(out=Tt[:, cj, :], in_=ps)

            # ---- V = X @ Wv^T -> bf16 [n, v].
            def emit_v(si):
                ps = ppsa.tile([P, D], F32, tag="psa", name=f"psV{si}")
                for ci in range(DC):
                    for vt in range(2):
                        nc.tensor.matmul(
                            ps[:, vt * 512 : (vt + 1) * 512],
                            Xt[:, ci, si * P : (si + 1) * P],
                            WvT[:, ci, vt * 512 : (vt + 1) * 512],
                            start=(ci == 0),
                            stop=(ci == DC - 1),
                            skip_group_check=True,
                        )
                nc.vector.tensor_copy(out=V[:, si, :], in_=ps)

            # ---- Per query-chunk: A -> exp -> weights out + E^T transpose.
            def emit_scores(mi):
                m0 = mi * P
                psa = [
                    ppsa.tile([P, D], F32, tag="psa", name=f"psa{mi}_{i}")
                    for i in range(2)
                ]
                for cj in range(DC):
                    for q in range(4):
                        nc.tensor.matmul(
                            psa[q // 2][:, (q % 2) * 512 : (q % 2 + 1) * 512],
                            Tt[:, cj, m0 : m0 + P],
                            Xt[:, cj, q * 512 : (q + 1) * 512],
                            start=(cj == 0),
                            stop=(cj == DC - 1),
                            skip_group_check=True,
                        )
                e32 = pe32.tile([P, S], F32, tag="e32", name=f"e32_{mi}")
                ebf = pebf.tile([P, S], BF16, tag="ebf", name=f"ebf_{mi}")
                acc = ptiny.tile([P, 2], F32, tag="acc", name=f"acc{mi}")
                # Per half: bf16 exp first, its E^T transpose dispatch right
                # behind it (so the transpose is not queued behind the other
                # half's PSUM wait on the ACT sequencer), then the fp32 exp
                # with the fused row-sum.
                for half in range(2):
                    sl = slice(half * D, (half + 1) * D)
                    nc.scalar.activation(
                        out=ebf[:, sl],
                        in_=psa[half],
                        func=mybir.ActivationFunctionType.Exp,
                        scale=float(SCALE),
                    )
                    nc.scalar.dma_start_transpose(
                        Et[:, half * DC : (half + 1) * DC, m0 : m0 + P],
                        ebf[:, sl],
                    )
                    nc.scalar.activation(
                        out=e32[:, sl],
                        in_=psa[half],
                        func=mybir.ActivationFunctionType.Exp,
                        scale=float(SCALE),
                        accum_out=acc[:, half : half + 1],
                    )
                ssum = ptiny.tile([P, 1], F32, tag="ssum", name=f"ssum{mi}")
                nc.vector.reduce_sum(out=ssum, in_=acc, axis=mybir.AxisListType.X)
                nc.vector.reciprocal(out=r_all[:, mi : mi + 1], in_=ssum)
                # weights output: E * (1/rowsum), streamed out in fp32.
                nc.vector.tensor_scalar_mul(e32, e32, r_all[:, mi : mi + 1])
                nc.gpsimd.dma_start(out=w_out[m0 : m0 + P, :], in_=e32)

            def emit_context(mi):
                m0 = mi * P
                ps = ppsa.tile([P, D], F32, tag="psa", name=f"psc{mi}")
                for si in range(SC):
                    for vt in range(2):
                        nc.tensor.matmul(
                            ps[:, vt * 512 : (vt + 1) * 512],
                            Et[:, si, m0 : m0 + P],
                            V[:, si, vt * 512 : (vt + 1) * 512],
                            start=(si == 0),
                            stop=(si == SC - 1),
                            skip_group_check=True,
                        )
                cst = pcst.tile([P, D], F32, tag="cst", name=f"cst{mi}")
                nc.vector.tensor_scalar_mul(cst, ps, r_all[:, mi : mi + 1])
                nc.gpsimd.dma_start(out=ctx_out[m0 : m0 + P, :], in_=cst)

            # PE stream: G, Tt, V, then the scores pipeline with the context
            # of the previous chunk interleaved (covers the ACT exp latency).
            for si in range(SC):
                emit_v(si)
            emit_scores(0)
            for mi in range(1, MC):
                emit_scores(mi)
                emit_context(mi - 1)
            emit_context(MC - 1)

    _dedupe_ldweights(nc)
    _split_excess_waits(nc)
    return nc


_NC_CACHE = None


def _get_program():
    global _NC_CACHE
    if _NC_CACHE is None:
        _NC_CACHE = _build_program()
    return _NC_CACHE


_EXEC_CACHE = None


def _get_executor(nc):
    """Cached sharded-jit executor mirroring bass2jax.run_bass_via_pjrt.

    run_bass_kernel_spmd re-jits (and re-lowers) on every call because its
    jitted body is a fresh closure each time; caching the jit here makes
    repeat kernel() calls several times faster. Falls back to the stock
    path in kernel() if anything about this environment changes.
    """
    global _EXEC_CACHE
    if _EXEC_CACHE is not None:
        return _EXEC_CACHE

    import jax
    from jax.experimental.shard_map import shard_map
    from jax.sharding import Mesh, PartitionSpec

    import concourse.bass2jax as bass2jax

    n_cores = 8
    bass2jax.install_neuronx_cc_hook()
    partition_name = nc.partition_id_tensor.name if nc.partition_id_tensor else None

    in_names, out_names, out_avals = [], [], []
    for alloc in nc.m.functions[0].allocations:
        if not isinstance(alloc, mybir.MemoryLocationSet):
            continue
        name = alloc.memorylocations[0].name
        if alloc.kind == "ExternalInput":
            if name != partition_name:
                in_names.append(name)
        elif alloc.kind == "ExternalOutput":
            out_names.append(name)
            out_avals.append(
                jax.core.ShapedArray(
                    tuple(alloc.tensor_shape), mybir.dt.np(alloc.dtype)
                )
            )
    n_params = len(in_names)
    all_names = in_names + out_names
    if partition_name is not None:
        all_names.append(partition_name)
    donate = tuple(range(n_params, n_params + len(out_names)))

    def _body(*args):
        operands = list(args)
        if partition_name is not None:
            operands.append(bass2jax.partition_id_tensor())
        outs = bass2jax._bass_exec_p.bind(
            *operands,
            out_avals=tuple(out_avals),
            in_names=tuple(all_names),
            out_names=tuple(out_names),
            lowering_input_output_aliases=(),
            sim_require_finite=True,
            sim_require_nnan=True,
            nc=nc,
        )
        return tuple(outs)

    devices = jax.devices()[:n_cores]
    if len(devices) < n_cores:
        raise RuntimeError(f"need {n_cores} devices, have {len(devices)}")
    mesh = Mesh(np.asarray(devices), ("core",))
    fn = jax.jit(
        shard_map(
            _body,
            mesh=mesh,
            in_specs=(PartitionSpec("core"),) * (n_params + len(out_names)),
            out_specs=(PartitionSpec("core"),) * len(out_names),
            check_rep=False,
        ),
        donate_argnums=donate,
        keep_unused=True,
    )

    def run(in_maps):
        per_core = [[m[n] for n in in_names] for m in in_maps]
        concat_in = [
            np.concatenate([per_core[c][i] for c in range(n_cores)], axis=0)
            for i in range(n_params)
        ]
        concat_zeros = [
            np.zeros((n_cores * a.shape[0], *a.shape[1:]), a.dtype)
            for a in out_avals
        ]
        out_arrs = fn(*concat_in, *concat_zeros)
        return [
            {
                name: np.asarray(out_arrs[i]).reshape(
                    n_cores, *out_avals[i].shape
                )[c]
                for i, name in enumerate(out_names)
            }
            for c in range(n_cores)
        ]

    _EXEC_CACHE = run
    return run


def kernel(X, W_q, W_k, W_v):
    X = np.asarray(X, dtype=np.float32)
    B = X.shape[0]
    n_cores = 8
    assert B * 2 == n_cores

    nc = _get_program()

    # Host-side packing: bf16 + layout transposes (no FLOPs).
    wqb = np.asarray(W_q, dtype=np.float32).astype(NP_BF16)
    wkb = np.asarray(W_k, dtype=np.float32).astype(NP_BF16)
    wvt = np.ascontiguousarray(np.asarray(W_v, dtype=np.float32).T).astype(NP_BF16)

    in_maps = []
    for core in range(n_cores):
        b, qh = core // 2, core % 2
        if qh == 0:
            xtb = np.ascontiguousarray(X[b].T).astype(NP_BF16)
            xt_prev = xtb
        else:
            xtb = np.ascontiguousarray(np.roll(xt_prev, -M, axis=1))
        in_maps.append({"xt": xtb, "wqb": wqb, "wkb": wkb, "wvt": wvt})

    try:
        results = _get_executor(nc)(in_maps)
    except Exception:
        results = run_bass_kernel_spmd(
            nc, in_maps, core_ids=list(range(n_cores))
        ).results

    context = np.empty((B, S, D), dtype=np.float32)
    weights = np.empty((B, S, S), dtype=np.float32)
    for core in range(n_cores):
        b, qh = core // 2, core % 2
        rows = slice(qh * M, (qh + 1) * M)
        context[b, rows, :] = results[core]["ctx_out"]
        w = results[core]["w_out"]
        weights[b, rows, :] = w if qh == 0 else np.roll(w, M, axis=1)
    return context, weights
